# revision 4
# baseline (speedup 1.0000x reference)
"""Coattention kernel for Trainium2 (Bass/Tile), data-parallel over batch on 8 cores.

Math per batch (all matrices 768x768, N==D==768):
  lo  = L @ Wc^T + bc            io  = I @ Wc^T + bc
  G2  = io^T
  S1  = lo^T... (reference: softmax((lo^T) @ (io^T)) etc.)  -- see reference.py
Implemented with PE matmuls (float32r), PE tile-transposes streamed into the
consumer matmul's stationary operand, ACT exp/scale softmax, DVE evacuations.
"""
import numpy as np

B = 32
D = 768
P = 128
NT = D // P  # 6
N_CORES = 8
NB = B // N_CORES  # batches per core

_cache = {}


def _build(nb, has_bias):
    import concourse.bass as bass
    import concourse.mybir as mybir
    import concourse.tile as tile
    from concourse import bacc
    from concourse.masks import make_identity
    from contextlib import ExitStack

    f32 = mybir.dt.float32
    f32r = mybir.dt.float32r
    Exp = mybir.ActivationFunctionType.Exp
    Copy = mybir.ActivationFunctionType.Copy

    nc = bacc.Bacc("TRN2", target_bir_lowering=False, debug=False)

    L_d = nc.dram_tensor("L", [nb, D, D], f32, kind="ExternalInput").ap()
    I_d = nc.dram_tensor("I", [nb, D, D], f32, kind="ExternalInput").ap()
    wct_d = nc.dram_tensor("wct", [D, D], f32, kind="ExternalInput").ap()
    wst_d = nc.dram_tensor("wst", [D, D], f32, kind="ExternalInput").ap()
    wxt_d = nc.dram_tensor("wxt", [D, D], f32, kind="ExternalInput").ap()
    bc_d = nc.dram_tensor("bc", [1, D], f32, kind="ExternalInput").ap()
    bs_d = nc.dram_tensor("bs", [1, D], f32, kind="ExternalInput").ap()
    bx_d = nc.dram_tensor("bx", [1, D], f32, kind="ExternalInput").ap()
    out_d = nc.dram_tensor("out", [nb, D, D], f32, kind="ExternalOutput").ap()

    NH = ((0, 512), (512, 768))  # psum-bank-aligned halves of the free dim

    with tile.TileContext(nc) as tc, ExitStack() as ctx:
        sb = ctx.enter_context(tc.tile_pool(name="sb", bufs=1))
        sml = ctx.enter_context(tc.tile_pool(name="sml", bufs=1))
        p_lin = ctx.enter_context(tc.tile_pool(name="p_lin", bufs=4))
        p_ts = ctx.enter_context(tc.tile_pool(name="p_ts", bufs=4))
        p_sm = ctx.enter_context(tc.tile_pool(name="p_sm", bufs=2))
        p_tiny = ctx.enter_context(tc.tile_pool(name="p_tiny", bufs=8))
        ps = ctx.enter_context(tc.tile_pool(name="ps", bufs=2, space="PSUM"))
        tps = ctx.enter_context(tc.tile_pool(name="tps", bufs=3, space="PSUM"))

        # --- identities ---
        ident = sml.tile([P, P], f32, tag="ident")
        make_identity(nc, ident[:])
        ident_r = sml.tile([P, P], f32r, tag="identr")
        nc.vector.tensor_copy(ident_r[:], ident[:])

        # --- weights: DMA f32 staging slabs, round-copy into resident f32r ---
        w_sb = {}
        for wname, wd in (("wc", wct_d), ("ws", wst_d), ("wx", wxt_d)):
            wt = sb.tile([P, NT, D], f32r, tag="w_" + wname)
            for k in range(NT):
                stg = p_lin.tile([P, D], f32, tag="lin")
                nc.sync.dma_start(stg[:], wd[k * P:(k + 1) * P, :])
                nc.vector.tensor_copy(wt[:, k], stg[:])
            w_sb[wname] = wt

        # --- bias broadcast tiles (built only when biases are nonzero) ---
        bcast = {}
        if has_bias:
            ones = sml.tile([1, P], f32, tag="ones")
            nc.gpsimd.memset(ones[:], 1.0)
            for bname, bd in (("bc", bc_d), ("bs", bs_d), ("bx", bx_d)):
                brow = sml.tile([1, D], f32, tag="brow")
                nc.sync.dma_start(brow[:], bd[:, :])
                bt = sml.tile([P, D], f32, tag="bb_" + bname)
                pt = ps.tile([P, D], f32, tag="mmout")
                for n0, n1 in NH:
                    nc.tensor.matmul(pt[:, n0:n1], ones[:, :], brow[:, n0:n1],
                                     start=True, stop=True)
                nc.vector.tensor_copy(bt[:], pt[:])
                bcast[bname] = bt

        def stream_T_tiles(src_slab_fn, e, dtype_in):
            """PE-transpose one [128,128] tile -> psum -> evac to f32r sbuf tile."""
            tp = tps.tile([P, P], f32, tag="tp")
            idn = ident if dtype_in == f32 else ident_r
            tpv = tp[:] if dtype_in == f32 else tp[:].bitcast(f32r)
            nc.tensor.matmul(tpv, src_slab_fn(e), idn[:], is_transpose=True,
                             start=True, stop=True)
            ts = p_ts.tile([P, P], f32r, tag="ts")
            nc.any.tensor_copy(ts[:], tp[:])
            return ts

        def mm_statT(src_tile_fn, mov, dtype_in=f32r):
            """For each m: psum[m] = sum_e T(src[m,e]) ... i.e. OUT = SRC @ MOV
            where SRC tiles come in natural orientation and are PE-transposed
            on the fly.  Yields (m, psum_tile)."""
            for m in range(NT):
                pt = ps.tile([P, D], f32, tag="mmout")
                for e in range(NT):
                    ts = stream_T_tiles(lambda ee: src_tile_fn(m, ee), e, dtype_in)
                    for n0, n1 in NH:
                        nc.tensor.matmul(pt[:, n0:n1], ts[:], mov[:, e, n0:n1],
                                         start=(e == 0), stop=(e == NT - 1))
                yield m, pt

        def mm_stat(stat, mov):
            """OUT = stat^T-object chain: psum[m] = sum_e stat[e,m]^T @ mov[e]."""
            for m in range(NT):
                pt = ps.tile([P, D], f32, tag="mmout")
                for e in range(NT):
                    for n0, n1 in NH:
                        nc.tensor.matmul(pt[:, n0:n1],
                                         stat[:, e, m * P:(m + 1) * P],
                                         mov[:, e, n0:n1],
                                         start=(e == 0), stop=(e == NT - 1))
                yield m, pt

        def evac(dst, m, pt, add=None, bias=None):
            """PSUM -> SBUF slab copy (rounds to dst dtype); optional residual add."""
            if add is not None:
                nc.vector.tensor_add(dst[:, m], pt[:], add[:, m])
            elif bias is not None:
                nc.vector.tensor_add(dst[:, m], pt[:], bias[:])
            else:
                nc.vector.tensor_copy(dst[:, m], pt[:])
            if add is not None and bias is not None:
                nc.vector.tensor_add(dst[:, m], dst[:, m], bias[:])

        def mat_T(dst, src):
            """dst = transpose(src) materialized, both [P,NT,D] f32r."""
            for j in range(NT):
                pt = ps.tile([P, D], f32, tag="mmout")
                for i in range(NT):
                    nc.tensor.matmul(pt[:, i * P:(i + 1) * P].bitcast(f32r),
                                     src[:, i, j * P:(j + 1) * P], ident_r[:],
                                     is_transpose=True, start=True, stop=True)
                nc.vector.tensor_copy(dst[:, j], pt[:])

        def softmax(dst, m, pt):
            """dst[:,m] = softmax over free dim of psum scores (no max-sub;
            scores are O(30) and exp is safe in fp32)."""
            e_sb = p_sm.tile([P, D], f32, tag="esl")
            sums = p_tiny.tile([P, 1], f32, tag="sums")
            nc.scalar.activation(e_sb[:], pt[:], Exp, accum_out=sums[:])
            rec = p_tiny.tile([P, 1], f32, tag="rec")
            nc.vector.reciprocal(rec[:], sums[:])
            nc.scalar.activation(dst[:, m], e_sb[:], Copy, scale=rec[:, 0:1])

        for b in range(nb):
            # tags pair roles with disjoint lifetimes (see design notes)
            t_lo = sb.tile([P, NT, D], f32r, tag="T1")
            t_io = sb.tile([P, NT, D], f32r, tag="T2")

            # s1/s2: lo = L @ WcT (+bc), io = I @ WcT (+bc); L/I streamed per slab
            for (src_d, dst) in ((L_d, t_lo), (I_d, t_io)):
                lslabs = []
                for m in range(NT):
                    lsl = p_lin.tile([P, D], f32, tag="lin")
                    nc.sync.dma_start(lsl[:], src_d[b, m * P:(m + 1) * P, :])
                    lslabs.append(lsl)
                for m, pt in mm_statT(
                        lambda mm, ee: lslabs[mm][:, ee * P:(ee + 1) * P],
                        w_sb["wc"], dtype_in=f32):
                    evac(dst, m, pt, bias=bcast.get("bc"))

            # s3: G2 = io^T materialized
            t_G2 = sb.tile([P, NT, D], f32r, tag="T7")
            mat_T(t_G2, t_io)

            # s4: S1 = lo^T... scores = matmul(stat=lo, mov=G2); A1 = softmax
            t_A1 = sb.tile([P, NT, D], f32r, tag="T3")
            for m, pt in mm_stat(t_lo, t_G2):
                softmax(t_A1, m, pt)

            # s5: co1 = io^T @ A1 = G2 @ A1  (stat=io, mov=A1)
            t_co1 = sb.tile([P, NT, D], f32r, tag="T4")
            for m, pt in mm_stat(t_io, t_A1):
                evac(t_co1, m, pt)

            # s6: co = co1^T @ WcT + lo^T (+bc): matmuls then lo-transposes into psum
            t_co = sb.tile([P, NT, D], f32r, tag="T5")
            for m in range(NT):
                pt = ps.tile([P, D], f32, tag="mmout")
                for e in range(NT):
                    for n0, n1 in NH:
                        nc.tensor.matmul(pt[:, n0:n1],
                                         t_co1[:, e, m * P:(m + 1) * P],
                                         w_sb["wc"][:, e, n0:n1],
                                         start=(e == 0), stop=False)
                for j in range(NT):
                    nc.tensor.matmul(pt[:, j * P:(j + 1) * P].bitcast(f32r),
                                     t_lo[:, j, m * P:(m + 1) * P], ident_r[:],
                                     is_transpose=True, start=False,
                                     stop=(j in (3, NT - 1)))
                evac(t_co, m, pt, bias=bcast.get("bc"))

            # s7: sp = co @ WsT (+bs)  (stream-T co tiles as stationary)
            t_sp = sb.tile([P, NT, D], f32r, tag="T1")
            for m, pt in mm_statT(
                    lambda mm, ee: t_co[:, mm, ee * P:(ee + 1) * P], w_sb["ws"]):
                evac(t_sp, m, pt, bias=bcast.get("bs"))

            # s8: spT materialized
            t_spT = sb.tile([P, NT, D], f32r, tag="T2")
            mat_T(t_spT, t_sp)

            # s9: S2 = sp^T... scores = matmul(stat=sp, mov=spT); A2 = softmax
            t_A2 = sb.tile([P, NT, D], f32r, tag="T3")
            for m, pt in mm_stat(t_sp, t_spT):
                softmax(t_A2, m, pt)

            # s10: sa1 = A2 @ co  (stream-T A2 tiles as stationary, mov=co)
            t_sa1 = sb.tile([P, NT, D], f32r, tag="T4")
            for m, pt in mm_statT(
                    lambda mm, ee: t_A2[:, mm, ee * P:(ee + 1) * P], t_co):
                evac(t_sa1, m, pt)

            # s11: sa = sa1^T @ WsT + co (+bs)
            t_sa = sb.tile([P, NT, D], f32r, tag="T6")
            for m, pt in mm_stat(t_sa1, w_sb["ws"]):
                evac(t_sa, m, pt, add=t_co, bias=bcast.get("bs"))

            # s12: xp = sa @ WxT (+bx)
            t_xp = sb.tile([P, NT, D], f32r, tag="T1")
            for m, pt in mm_statT(
                    lambda mm, ee: t_sa[:, mm, ee * P:(ee + 1) * P], w_sb["wx"]):
                evac(t_xp, m, pt, bias=bcast.get("bx"))

            # s13: S3 = matmul(stat=xp, mov=G2); A3 = softmax
            t_A3 = sb.tile([P, NT, D], f32r, tag="T3")
            for m, pt in mm_stat(t_xp, t_G2):
                softmax(t_A3, m, pt)

            # s14: xa1 = A3 @ G2
            t_xa1 = sb.tile([P, NT, D], f32r, tag="T4")
            for m, pt in mm_statT(
                    lambda mm, ee: t_A3[:, mm, ee * P:(ee + 1) * P], t_G2):
                evac(t_xa1, m, pt)

            # s15: out = xa1^T @ WxT + sa (+bx) -> DMA per slab
            for m, pt in mm_stat(t_xa1, w_sb["wx"]):
                osl = p_sm.tile([P, D], f32, tag="outsl")
                nc.vector.tensor_add(osl[:], pt[:], t_sa[:, m])
                if has_bias:
                    nc.vector.tensor_add(osl[:], osl[:], bcast["bx"][:])
                nc.sync.dma_start(out_d[b, m * P:(m + 1) * P, :], osl[:])

    nc.finalize()
    return nc


def _get_program(nb, has_bias):
    key = (nb, has_bias)
    if key not in _cache:
        _cache[key] = _build(nb, has_bias)
    return _cache[key]


def kernel(language_output, image_output, Wc, bc, Ws, bs, Wx, bx,
           _n_cores=N_CORES, _nb=None):
    from concourse import bass_utils

    L = np.ascontiguousarray(np.asarray(language_output, dtype=np.float32))
    I = np.ascontiguousarray(np.asarray(image_output, dtype=np.float32))
    wct = np.ascontiguousarray(np.asarray(Wc, dtype=np.float32).T)
    wst = np.ascontiguousarray(np.asarray(Ws, dtype=np.float32).T)
    wxt = np.ascontiguousarray(np.asarray(Wx, dtype=np.float32).T)
    bc_ = np.asarray(bc, dtype=np.float32).reshape(1, D)
    bs_ = np.asarray(bs, dtype=np.float32).reshape(1, D)
    bx_ = np.asarray(bx, dtype=np.float32).reshape(1, D)
    has_bias = bool(np.any(bc_) or np.any(bs_) or np.any(bx_))

    batch = L.shape[0]
    n_cores = _n_cores
    nb = _nb if _nb is not None else batch // n_cores
    assert nb * n_cores == batch

    nc = _get_program(nb, has_bias)

    in_maps = []
    for c in range(n_cores):
        sl = slice(c * nb, (c + 1) * nb)
        in_maps.append({
            "L": L[sl], "I": I[sl],
            "wct": wct, "wst": wst, "wxt": wxt,
            "bc": bc_, "bs": bs_, "bx": bx_,
        })
    res = bass_utils.run_bass_kernel_spmd(nc, in_maps, list(range(n_cores)))
    out = np.empty((batch, D, D), dtype=np.float32)
    for c in range(n_cores):
        out[c * nb:(c + 1) * nb] = res.results[c]["out"]
    return out
